# revision 9
# baseline (speedup 1.0000x reference)
"""Mistral attention (B=2, S=2048, D=4096, H=32, KVH=8, HD=128) on 8 trn2 cores.

Sharding: core c -> (batch b = c//4, head-group g = c%4).
Each core computes q/k/v projections for its 8 Q heads + 2 KV heads of one
batch, RoPE, causal attention, and a row-parallel partial o_proj
[2048, 4096]. Host sums the 4 partials per batch. No collectives.

All matmuls run as float32r (full-rate fp32, ~1e-4 rel err).
Attention is computed in transposed orientation: scoresT[keys, qtok] with
keys on partitions, so softmax uses an unstable exp (logits are O(10) for
this data distribution; exp is fp32-safe), the key-sum is a ones-matmul,
and AV^T produces attn_out^T which feeds o_proj directly as the stationary
operand. For the causal variant, attention for query block t is fused right
after the projections of token block t (its K/V prefix is already on-chip).
"""

import os
import sys

for _p in ("/opt/trn_rl_repo",):
    if _p not in sys.path:
        sys.path.insert(0, _p)

import numpy as np

import concourse.bass as bass
import concourse.tile as tile
from concourse import bacc, mybir
from concourse.bass_utils import run_bass_kernel_spmd

F32 = mybir.dt.float32
F32R = mybir.dt.float32r
EXP = mybir.ActivationFunctionType.Exp

B, S, D = 2, 2048, 4096
H, KVH, HD = 32, 8, 128
SCALE = HD ** -0.5
NCORES = 8

QH = H // 4              # 8 q heads per core
QCOLS = QH * HD          # 1024
KCOLS = (KVH // 4) * HD  # 256 (2 kv heads per core)
TOK = S

NEG = -1e9

_PROGRAMS = {}


def _build_program(variant: str):
    """variant: 'causal' | 'zero' | 'general'"""
    nc = bacc.Bacc("TRN2", target_bir_lowering=False, debug=False)

    hT = nc.dram_tensor("hT", [4, 2, 128, 16 * 512], F32R, kind="ExternalInput").ap()
    wq = nc.dram_tensor("wq", [8, 2, 128, 16 * 128], F32R, kind="ExternalInput").ap()
    wk = nc.dram_tensor("wk", [2, 2, 128, 16 * 128], F32R, kind="ExternalInput").ap()
    wv = nc.dram_tensor("wv", [2, 2, 128, 16 * 128], F32R, kind="ExternalInput").ap()
    wo = nc.dram_tensor("wo", [8, 8, 128, 512], F32R, kind="ExternalInput").ap()
    cosT = nc.dram_tensor("cosT", [HD, TOK], F32, kind="ExternalInput").ap()
    sinTr = nc.dram_tensor("sinTr", [HD, TOK], F32, kind="ExternalInput").ap()
    ident = nc.dram_tensor("ident", [128, 128], F32R, kind="ExternalInput").ap()
    ones = nc.dram_tensor("ones", [128, 1], F32R, kind="ExternalInput").ap()
    if variant == "causal":
        maskT = nc.dram_tensor("maskT", [128, 4 * 512], F32, kind="ExternalInput").ap()
    elif variant == "general":
        maskT = nc.dram_tensor("maskT", [S, S], F32, kind="ExternalInput").ap()
    else:
        maskT = None
    out = nc.dram_tensor("out", [TOK, D], F32, kind="ExternalOutput").ap()

    attnT_spill = nc.dram_tensor("attnT_spill", [QCOLS, TOK], F32R).ap()
    if variant != "causal":
        qT_spill = nc.dram_tensor("qT_spill", [QCOLS, TOK], F32R).ap()

    NTH = 4
    THW = TOK // NTH         # 512
    NCH = D // 128           # 32 contraction chunks
    NCB = (QCOLS + 2 * KCOLS) // 128  # 12: 0-7 q, 8-9 k, 10-11 v

    with tile.TileContext(nc) as tc:
        with tc.tile_pool(name="per", bufs=1) as per, \
             tc.tile_pool(name="wrk", bufs=2) as wrk, \
             tc.tile_pool(name="one", bufs=1) as one, \
             tc.tile_pool(name="ps", bufs=2, space="PSUM") as psp:

            ident_sb = per.tile([128, 128], F32R, tag="ident")
            ones_sb = per.tile([128, 1], F32R, tag="ones")
            kT_sb = per.tile([HD, 2 * TOK], F32R, tag="kT")
            V_sb = per.tile([128, (TOK // 128) * KCOLS], F32R, tag="V")
            nc.sync.dma_start(ident_sb[:], ident[:])
            nc.sync.dma_start(ones_sb[:], ones[:])
            if variant == "causal":
                mask_sb = per.tile([128, 4 * 512], F32, tag="mask")
                nc.sync.dma_start(mask_sb[:], maskT[:])

            def attention(h, qb, qT_ap):
                """scoresT/softmax/AV^T for q heads h, query block qb.
                qT_ap: [128, 512] f32r SBUF AP for this head/block.
                Emits DMA of normalized attn_out^T to attnT_spill."""
                kv = h // (QH // 2)
                qs = qb * 512
                nkb = 4 * qb + 4 if variant == "causal" else TOK // 128
                att_ps = psp.tile([128, 512], F32, tag="aux")
                sum_ps = psp.tile([1, 512], F32, tag="sum")
                for kb in range(nkb):
                    s_ps = psp.tile([128, 512], F32, tag="pb")
                    nc.tensor.matmul(
                        s_ps[:],
                        kT_sb[:, kv * TOK + kb * 128: kv * TOK + (kb + 1) * 128],
                        qT_ap,
                        start=True, stop=True)
                    exp_in = s_ps
                    if variant == "causal" and kb >= 4 * qb:
                        o = kb - 4 * qb
                        msk = wrk.tile([128, 512], F32, tag="m1")
                        nc.vector.tensor_add(
                            msk[:], s_ps[:], mask_sb[:, o * 512:(o + 1) * 512])
                        exp_in = msk
                    elif variant == "general":
                        mt = wrk.tile([128, 512], F32, tag="mt")
                        nc.sync.dma_start(
                            mt[:], maskT[kb * 128:(kb + 1) * 128, qs:qs + 512])
                        msk = wrk.tile([128, 512], F32, tag="m1")
                        nc.vector.tensor_add(msk[:], s_ps[:], mt[:])
                        exp_in = msk
                    expT = wrk.tile([128, 512], F32R, tag="expT")
                    nc.scalar.activation(expT[:], exp_in[:], EXP, scale=float(SCALE))
                    nc.tensor.matmul(
                        att_ps[:],
                        V_sb[:, kb * KCOLS + kv * 128: kb * KCOLS + (kv + 1) * 128],
                        expT[:],
                        start=(kb == 0), stop=(kb == nkb - 1))
                    nc.tensor.matmul(
                        sum_ps[:], ones_sb[:], expT[:],
                        start=(kb == 0), stop=(kb == nkb - 1))
                atu = wrk.tile([128, 512], F32, tag="atu")
                nc.scalar.copy(atu[:], att_ps[:])
                recip = wrk.tile([1, 512], F32, tag="rcp")
                nc.vector.reciprocal(recip[:], sum_ps[:])
                rb = wrk.tile([128, 512], F32, tag="m2")
                nc.gpsimd.partition_broadcast(rb[:], recip[:])
                at2 = wrk.tile([128, 512], F32R, tag="vT")
                nc.vector.tensor_mul(at2[:], atu[:], rb[:])
                nc.sync.dma_start(
                    attnT_spill[h * 128:(h + 1) * 128, qs:qs + 512], at2[:])

            # ============ Phase A (+fused attention for causal) ============
            for th in range(NTH):
                ts = th * THW
                # hidden^T block [D, 512] as 8 sub-tiles of 4 D-chunks
                hts = []
                for j in range(8):
                    t = one.tile([128, 4 * THW], F32R, tag=f"hT{j}")
                    half, jj = divmod(j, 4)
                    nc.sync.dma_start(
                        t[:], hT[th, half, :, jj * 2048:(jj + 1) * 2048])
                    hts.append(t)
                cos_t = wrk.tile([HD, THW], F32, tag="cos")
                sin_t = wrk.tile([HD, THW], F32, tag="sin")
                nc.sync.dma_start(cos_t[:], cosT[:, ts:ts + THW])
                nc.sync.dma_start(sin_t[:], sinTr[:, ts:ts + THW])

                qT_blk = one.tile([128, QH * 512], F32R, tag="qTb")

                for cb in range(NCB):
                    if cb < 8:
                        wsrc, widx = wq, cb
                    elif cb < 10:
                        wsrc, widx = wk, cb - 8
                    else:
                        wsrc, widx = wv, cb - 10
                    ps = psp.tile([128, THW], F32, tag="pa")
                    for half in range(2):
                        w_sb = wrk.tile([128, (NCH // 2) * 128], F32R, tag="w")
                        nc.sync.dma_start(w_sb[:], wsrc[widx, half])
                        for i in range(NCH // 2):
                            ic = half * (NCH // 2) + i
                            t = hts[ic // 4]
                            nc.tensor.matmul(
                                ps[:],
                                w_sb[:, i * 128:(i + 1) * 128],
                                t[:, (ic % 4) * THW:(ic % 4 + 1) * THW],
                                start=(half == 0 and i == 0),
                                stop=(half == 1 and i == NCH // 2 - 1),
                            )
                    if cb < 10:
                        # RoPE: out = x*cos + swap_halves(x)*sin_signed
                        m1 = wrk.tile([128, THW], F32, tag="m1")
                        nc.vector.tensor_mul(m1[:], ps[:], cos_t[:])
                        m2 = wrk.tile([128, THW], F32, tag="m2")
                        nc.vector.tensor_mul(m2[0:64, :], ps[64:128, :], sin_t[0:64, :])
                        nc.vector.tensor_mul(m2[64:128, :], ps[0:64, :], sin_t[64:128, :])
                        if cb < 8:
                            nc.vector.tensor_add(
                                qT_blk[:, cb * 512:(cb + 1) * 512], m1[:], m2[:])
                        else:
                            kv = cb - 8
                            nc.vector.tensor_add(
                                kT_sb[:, kv * TOK + ts: kv * TOK + ts + THW],
                                m1[:], m2[:])
                    else:
                        kv = cb - 10
                        vT = wrk.tile([128, THW], F32R, tag="vT")
                        nc.scalar.copy(vT[:], ps[:])
                        for j in range(THW // 128):
                            tb = th * (THW // 128) + j
                            pt = psp.tile([128, 128], F32R, tag="aux")
                            nc.tensor.transpose(
                                pt[:], vT[:, j * 128:(j + 1) * 128], ident_sb[:])
                            nc.scalar.copy(
                                V_sb[:, tb * KCOLS + kv * 128:
                                     tb * KCOLS + (kv + 1) * 128],
                                pt[:])

                if variant == "causal":
                    for h in range(QH):
                        attention(h, th, qT_blk[:, h * 512:(h + 1) * 512])
                else:
                    nc.sync.dma_start(
                        qT_spill[:, ts:ts + THW]
                        .rearrange("(i p) t -> p i t", p=128),
                        qT_blk[:].rearrange("p (i t) -> p i t", i=QH),
                    )

            if variant != "causal":
                for h in range(QH):
                    for qb in range(4):
                        qT_t = wrk.tile([128, 512], F32R, tag="qTs")
                        nc.sync.dma_start(
                            qT_t[:],
                            qT_spill[h * 128:(h + 1) * 128, qb * 512:(qb + 1) * 512])
                        attention(h, qb, qT_t[:])

            # ================= Phase C: o_proj partial =================
            for qg in range(2):
                ags = []
                for h in range(QH):
                    a = one.tile([128, 1024], F32R, tag=f"hT{h}")
                    nc.sync.dma_start(
                        a[:],
                        attnT_spill[h * 128:(h + 1) * 128,
                                    qg * 1024:(qg + 1) * 1024])
                    ags.append(a)
                for nb in range(D // 512):
                    wo_sb = wrk.tile([128, QH * 512], F32R, tag="w")
                    for hc in range(QH):
                        nc.sync.dma_start(
                            wo_sb[:, hc * 512:(hc + 1) * 512], wo[nb, hc])
                    for q2 in range(8):
                        qtb = qg * 8 + q2
                        o_ps = psp.tile([128, 512], F32, tag="pa")
                        for hc in range(QH):
                            nc.tensor.matmul(
                                o_ps[:],
                                ags[hc][:, q2 * 128:(q2 + 1) * 128],
                                wo_sb[:, hc * 512:(hc + 1) * 512],
                                start=(hc == 0), stop=(hc == QH - 1))
                        ot = wrk.tile([128, 512], F32, tag="ot")
                        nc.scalar.copy(ot[:], o_ps[:])
                        nc.sync.dma_start(
                            out[qtb * 128:(qtb + 1) * 128, nb * 512:(nb + 1) * 512],
                            ot[:])

    nc.compile()
    return nc


def _get_program(variant: str):
    if variant not in _PROGRAMS:
        _PROGRAMS[variant] = _build_program(variant)
    return _PROGRAMS[variant]


def _detect_variant(mask: np.ndarray) -> str:
    m = mask.reshape(mask.shape[-2], mask.shape[-1])
    if not m.any():
        return "zero"
    causal = np.where(
        np.tril(np.ones((S, S), dtype=bool)), np.float32(0.0), np.float32(NEG))
    if np.array_equal(m, causal):
        return "causal"
    return "general"


def kernel(hidden_states, cos, sin, attention_mask, Wq, Wk, Wv, Wo):
    hidden_states = np.asarray(hidden_states, dtype=np.float32)
    cos = np.asarray(cos, dtype=np.float32)
    sin = np.asarray(sin, dtype=np.float32)
    attention_mask = np.asarray(attention_mask, dtype=np.float32)
    Wq = np.asarray(Wq, dtype=np.float32)
    Wk = np.asarray(Wk, dtype=np.float32)
    Wv = np.asarray(Wv, dtype=np.float32)
    Wo = np.asarray(Wo, dtype=np.float32)

    variant = _detect_variant(attention_mask)
    nc = _get_program(variant)

    ident = np.eye(128, dtype=np.float32)
    ones = np.ones((128, 1), dtype=np.float32)

    if variant == "causal":
        i = np.arange(128)[:, None]
        j = np.arange(512)[None, :]
        strips = [
            np.where(i <= j - o * 128, np.float32(0.0), np.float32(NEG / SCALE))
            for o in range(4)
        ]
        maskT = np.concatenate(strips, axis=1).astype(np.float32)
    elif variant == "general":
        m = attention_mask.reshape(S, S)
        maskT = np.ascontiguousarray(m.T / np.float32(SCALE))
    else:
        maskT = None

    per_batch = {}
    for b in range(B):
        sT = np.ascontiguousarray(sin[b].T)
        sinTr = np.concatenate([-sT[:64], sT[64:]], axis=0)
        hid = hidden_states[b]  # [2048, 4096]
        hT_t = np.ascontiguousarray(
            hid.reshape(4, 512, 2, 16, 128).transpose(0, 2, 4, 3, 1)
            .reshape(4, 2, 128, 16 * 512))
        per_batch[b] = (hT_t, np.ascontiguousarray(cos[b].T),
                        np.ascontiguousarray(sinTr))

    def _tile_w(W):  # [4096, C] -> [C//128, 2, 128, 2048]
        C = W.shape[1]
        return np.ascontiguousarray(
            W.reshape(2, 16, 128, C // 128, 128).transpose(3, 0, 2, 1, 4)
            .reshape(C // 128, 2, 128, 16 * 128))

    in_maps = []
    for c in range(NCORES):
        b, g = divmod(c, 4)
        hT_t, cosT, sinTr = per_batch[b]
        wo_c = Wo[g * QCOLS:(g + 1) * QCOLS, :]  # [1024, 4096]
        wo_t = np.ascontiguousarray(
            wo_c.reshape(8, 128, 8, 512).transpose(2, 0, 1, 3))
        im = {
            "hT": hT_t,
            "wq": _tile_w(Wq[:, g * QCOLS:(g + 1) * QCOLS]),
            "wk": _tile_w(Wk[:, g * KCOLS:(g + 1) * KCOLS]),
            "wv": _tile_w(Wv[:, g * KCOLS:(g + 1) * KCOLS]),
            "wo": wo_t,
            "cosT": cosT,
            "sinTr": sinTr,
            "ident": ident,
            "ones": ones,
        }
        if maskT is not None:
            im["maskT"] = maskT
        in_maps.append(im)

    trace = bool(os.environ.get("KERNEL_TRACE"))
    res = run_bass_kernel_spmd(nc, in_maps, core_ids=list(range(NCORES)),
                               trace=trace)
    if trace:
        print(f"HW exec time: {res.exec_time_ns} ns")

    out = np.empty((B, S, D), dtype=np.float32)
    for b in range(B):
        acc = np.zeros((S, D), dtype=np.float64)
        for g in range(4):
            acc += res.results[4 * b + g]["out"]
        out[b] = acc.astype(np.float32)
    return out
